# revision 10
# baseline (speedup 1.0000x reference)
"""Trainium2 Bass kernel for nn_CrossAttention (B=4, T=2048, 1024 dims, 16 heads).

Sharding: 8 cores = 4 batches x 2 head-groups (8 heads each).
Per core (SPMD, same program, different data):
  phase 1: qT = Wq_g^T @ query_b^T + bq  (f32r, [512, 2048], head-dim on partitions)
           kT = Wk_g^T @ context_b^T + bk
           v  = context_b @ Wv_g + bv    ([2048, 512], with ones column per head)
  phase 2: per head h: scoresT = kT_h^T(stationary) x qT_h(moving) -> PSUM [tc, tq]
           probsT = exp(scoresT)  (ScalarE, no max subtraction: |scores| < 3)
           [attnT; denom] += [v_h, 1]^T x probsT  (PSUM accumulate over tc tiles)
           attnT_norm = attnT * (1/denom)  (broadcast via SWDGE DMA)
  phase 3: out_partial = attnT_norm^T @ Wo_g  -> DRAM
Host: out[b] = partial[2b] + partial[2b+1] + bo.   1/sqrt(D) folded into Wq/bq.
"""
import numpy as np
import concourse.bacc as bacc
import concourse.mybir as mybir
from concourse.tile import TileContext
from concourse.bass_utils import run_bass_kernel_spmd

N_CORES = 8
P = 128
F = 1024            # query/context feature dim
KF = F // P         # 8 f-tiles
HD = 512            # head-group hidden dim (8 heads x 64)
HT = HD // P        # 4 h-tiles
TQ = TC = 2048
NH = 8              # heads per core
D = 64
CH = 1024           # tq chunk for phase 2/3
NCH = TQ // CH      # 2
TCT = TC // P       # 16 tc tiles
F32 = mybir.dt.float32
F32R = mybir.dt.float32r

_NC_CACHE = None


def build_kernel(debug=False):
    nc = bacc.Bacc("TRN2", target_bir_lowering=False, debug=False, num_devices=N_CORES)

    qt_d = nc.dram_tensor("qt", [F, TQ], F32R, kind="ExternalInput")     # query[b].T
    ct_d = nc.dram_tensor("ct", [F, TC], F32R, kind="ExternalInput")     # context[b].T
    wq_d = nc.dram_tensor("wq", [F, HD], F32R, kind="ExternalInput")     # pre-scaled 1/8
    wk_d = nc.dram_tensor("wk", [F, HD], F32R, kind="ExternalInput")
    wv_d = nc.dram_tensor("wv", [F, HD], F32R, kind="ExternalInput")
    wo_d = nc.dram_tensor("wo", [HD, 1024], F32R, kind="ExternalInput")
    bq_d = nc.dram_tensor("bq", [HT, P], F32, kind="ExternalInput")      # pre-scaled 1/8
    bk_d = nc.dram_tensor("bk", [HT, P], F32, kind="ExternalInput")
    bv_d = nc.dram_tensor("bv", [1, HD], F32R, kind="ExternalInput")
    out_d = nc.dram_tensor("out", [TQ, 1024], F32, kind="ExternalOutput")
    scr_d = nc.dram_tensor("scr", [NCH, NH, CH], F32)  # denom-recip bounce for broadcast
    if debug:
        qT_dump = nc.dram_tensor("qT_dump", [P, HT, TQ], F32, kind="ExternalOutput")
        kT_dump = nc.dram_tensor("kT_dump", [P, HT, TC], F32, kind="ExternalOutput")
        vv_dump = nc.dram_tensor("vv_dump", [P, TCT, NH, D + 1], F32, kind="ExternalOutput")
        at_dump = nc.dram_tensor("at_dump", [NCH, P, HT, CH], F32, kind="ExternalOutput")
        pb_dump = nc.dram_tensor("pb_dump", [P, CH], F32, kind="ExternalOutput")
        pa_dump = nc.dram_tensor("pa_dump", [D + 1, CH], F32, kind="ExternalOutput")
        bc_dump = nc.dram_tensor("bc_dump", [D, CH], F32, kind="ExternalOutput")

    with TileContext(nc) as tc:
        with tc.tile_pool(name="consts", bufs=1) as consts, \
             tc.tile_pool(name="qkv", bufs=1) as qkv, \
             tc.tile_pool(name="ps_proj", bufs=2, space="PSUM") as ps_proj, \
             tc.tile_pool(name="ps_scores", bufs=2, space="PSUM") as ps_scores, \
             tc.tile_pool(name="ps_attn", bufs=1, space="PSUM") as ps_attn:

            ones = consts.tile([1, P], F32R)
            nc.vector.memset(ones.bitcast(F32), 1.0)
            bv_sb = consts.tile([1, HD], F32R)
            nc.sync.dma_start(out=bv_sb, in_=bv_d[:, :])
            bq_sb = consts.tile([P, HT], F32)
            nc.sync.dma_start(out=bq_sb, in_=bq_d.rearrange("m p -> p m"))
            bk_sb = consts.tile([P, HT], F32)
            nc.sync.dma_start(out=bk_sb, in_=bk_d.rearrange("m p -> p m"))

            qT = qkv.tile([P, HT, TQ], F32R)        # [h-dim, tq]
            kT = qkv.tile([P, HT, TC], F32R)        # [h-dim, tc]
            vv = qkv.tile([P, TCT, NH, D + 1], F32R)  # per head: [v | ones]
            nc.vector.memset(vv.bitcast(F32)[:, :, :, D:D + 1], 1.0)

            # ---------------- phase 1: projections ----------------
            with tc.tile_pool(name="wts", bufs=1) as wts, \
                 tc.tile_pool(name="chunks", bufs=2) as chunks:
                wq = wts.tile([P, KF, HD], F32R)
                nc.sync.dma_start(out=wq, in_=wq_d.rearrange("(k p) h -> p k h", p=P))
                wk = wts.tile([P, KF, HD], F32R)
                nc.sync.dma_start(out=wk, in_=wk_d.rearrange("(k p) h -> p k h", p=P))
                wv = wts.tile([P, KF, HD], F32R)
                nc.sync.dma_start(out=wv, in_=wv_d.rearrange("(k p) h -> p k h", p=P))

                qt_r = qt_d.rearrange("(k p) t -> p k t", p=P)
                ct_r = ct_d.rearrange("(k p) t -> p k t", p=P)

                # qT = Wq^T @ queryT (+ bq per-partition on evacuation)
                for n in range(TQ // 512):
                    qc = chunks.tile([P, KF, 512], F32R, tag="chunk")
                    nc.sync.dma_start(out=qc, in_=qt_r[:, :, n * 512:(n + 1) * 512])
                    for m in range(HT):
                        ps = ps_proj.tile([P, 512], F32)
                        for k in range(KF):
                            nc.tensor.matmul(ps, wq[:, k, m * P:(m + 1) * P], qc[:, k, :],
                                             start=(k == 0), stop=(k == KF - 1))
                        nc.vector.tensor_scalar_add(
                            out=qT[:, m, n * 512:(n + 1) * 512], in0=ps,
                            scalar1=bq_sb[:, m:m + 1])
                # kT and v from shared contextT chunks
                for n in range(TC // 512):
                    cc = chunks.tile([P, KF, 512], F32R, tag="chunk")
                    nc.sync.dma_start(out=cc, in_=ct_r[:, :, n * 512:(n + 1) * 512])
                    for m in range(HT):
                        ps = ps_proj.tile([P, 512], F32)
                        for k in range(KF):
                            nc.tensor.matmul(ps, wk[:, k, m * P:(m + 1) * P], cc[:, k, :],
                                             start=(k == 0), stop=(k == KF - 1))
                        nc.vector.tensor_scalar_add(
                            out=kT[:, m, n * 512:(n + 1) * 512], in0=ps,
                            scalar1=bk_sb[:, m:m + 1])
                    for tl in range(4):
                        i = n * 4 + tl
                        ps = ps_proj.tile([P, 512], F32)
                        for k in range(KF):
                            nc.tensor.matmul(ps, cc[:, k, tl * P:(tl + 1) * P], wv[:, k, :],
                                             start=(k == 0), stop=False)
                        nc.tensor.matmul(ps, ones[0:1, 0:P], bv_sb[0:1, :],
                                         start=False, stop=True)
                        nc.vector.tensor_copy(
                            out=vv[:, i, :, 0:D],
                            in_=ps.rearrange("p (h d) -> p h d", h=NH))

            # ---------------- phase 2 + 3 ----------------
            with tc.tile_pool(name="wop", bufs=1) as wop, \
                 tc.tile_pool(name="probs", bufs=2) as probs, \
                 tc.tile_pool(name="attnsb", bufs=2) as attnsb, \
                 tc.tile_pool(name="norm", bufs=2) as norm, \
                 tc.tile_pool(name="outsb", bufs=2) as outsb:
                wo = wop.tile([P, HT, 1024], F32R)
                nc.sync.dma_start(out=wo, in_=wo_d.rearrange("(m p) o -> p m o", p=P))

                if debug:
                    nc.sync.dma_start(out=qT_dump[:, :, :], in_=qT.bitcast(F32))
                    nc.sync.dma_start(out=kT_dump[:, :, :], in_=kT.bitcast(F32))
                    nc.sync.dma_start(out=vv_dump[:, :, :, :], in_=vv.bitcast(F32))
                for c in range(NCH):
                    # phase 2: attention for this tq chunk
                    at = attnsb.tile([P, HT, CH], F32R, tag="attnsb")
                    for h in range(NH):
                        ht, hp = h // 2, (h % 2) * D
                        pa = ps_attn.tile([D + 1, CH], F32)
                        for i in range(TCT):
                            ss = ps_scores.tile([P, CH], F32)
                            for n in range(CH // 512):
                                nc.tensor.matmul(
                                    ss[:, n * 512:(n + 1) * 512],
                                    kT[hp:hp + D, ht, i * P:(i + 1) * P],
                                    qT[hp:hp + D, ht,
                                       c * CH + n * 512:c * CH + (n + 1) * 512],
                                    start=True, stop=True)
                            pb = probs.tile([P, CH], F32R)
                            nc.scalar.activation(pb, ss, mybir.ActivationFunctionType.Exp)
                            if debug and c == 0 and h == 0 and i == 0:
                                nc.sync.dma_start(out=pb_dump[:, :], in_=pb.bitcast(F32))
                            for n in range(CH // 512):
                                nc.tensor.matmul(
                                    pa[:, n * 512:(n + 1) * 512],
                                    vv[:, i, h, :],
                                    pb[:, n * 512:(n + 1) * 512],
                                    start=(i == 0), stop=(i == TCT - 1))
                        if debug and c == 0 and h == 0:
                            pa_sb = norm.tile([D + 1, CH], F32, tag="pa_dbg")
                            nc.vector.tensor_copy(out=pa_sb, in_=pa)
                            nc.sync.dma_start(out=pa_dump[:, :], in_=pa_sb)
                        # normalize: recip of denom row, broadcast, multiply
                        rc = norm.tile([D + 1, CH], F32, tag="rc")
                        nc.vector.reciprocal(out=rc[D:D + 1, :], in_=pa[D:D + 1, :])
                        bc = norm.tile([D, CH], F32, tag="bc")
                        nc.sync.dma_start(out=scr_d[c, h:h + 1, :], in_=rc[D:D + 1, :])
                        nc.gpsimd.dma_start(
                            out=bc, in_=scr_d[c, h:h + 1, :].to_broadcast([D, CH]))
                        if debug and c == 0 and h == 0:
                            nc.sync.dma_start(out=bc_dump[:, :], in_=bc)
                        if h % 2 == 0:
                            nc.vector.tensor_mul(out=at[0:D, ht, :], in0=pa[0:D, :], in1=bc)
                        else:
                            tmp = norm.tile([D, CH], F32R, tag="tmp")
                            nc.vector.tensor_mul(out=tmp, in0=pa[0:D, :], in1=bc)
                            nc.gpsimd.dma_start(out=at[D:P, ht, :], in_=tmp)
                    if debug:
                        nc.sync.dma_start(out=at_dump[c], in_=at.bitcast(F32))
                    # phase 3: output projection for this chunk
                    for t in range(CH // P):
                        ot = outsb.tile([P, 1024], F32)
                        for o in range(2):
                            po = ps_proj.tile([P, 512], F32, tag="ps")
                            for m in range(HT):
                                nc.tensor.matmul(po, at[:, m, t * P:(t + 1) * P],
                                                 wo[:, m, o * 512:(o + 1) * 512],
                                                 start=(m == 0), stop=(m == HT - 1))
                            nc.vector.tensor_copy(out=ot[:, o * 512:(o + 1) * 512], in_=po)
                        nc.sync.dma_start(
                            out=out_d[c * CH + t * P:c * CH + (t + 1) * P, :], in_=ot)

    nc.compile()
    return nc


def make_in_maps(query, context, Wq, bq, Wk, bk, Wv, bv, Wo, bo):
    query = np.asarray(query, np.float32)
    context = np.asarray(context, np.float32)
    Wq = np.asarray(Wq, np.float32); bq = np.asarray(bq, np.float32)
    Wk = np.asarray(Wk, np.float32); bk = np.asarray(bk, np.float32)
    Wv = np.asarray(Wv, np.float32); bv = np.asarray(bv, np.float32)
    Wo = np.asarray(Wo, np.float32)

    in_maps = []
    for c in range(N_CORES):
        b, g = c // 2, c % 2
        sl = slice(g * HD, (g + 1) * HD)
        in_maps.append({
            "qt": np.ascontiguousarray(query[b].T),
            "ct": np.ascontiguousarray(context[b].T),
            "wq": np.ascontiguousarray(Wq[:, sl] * 0.125),
            "wk": np.ascontiguousarray(Wk[:, sl]),
            "wv": np.ascontiguousarray(Wv[:, sl]),
            "wo": np.ascontiguousarray(Wo[sl, :]),
            "bq": np.ascontiguousarray((bq[sl] * 0.125).reshape(HT, P)),
            "bk": np.ascontiguousarray(bk[sl].reshape(HT, P)),
            "bv": np.ascontiguousarray(bv[sl].reshape(1, HD)),
        })
    return in_maps


def kernel(query, context, Wq, bq, Wk, bk, Wv, bv, Wo, bo):
    global _NC_CACHE
    if _NC_CACHE is None:
        _NC_CACHE = build_kernel()
    nc = _NC_CACHE
    bo = np.asarray(bo, np.float32)

    in_maps = make_in_maps(query, context, Wq, bq, Wk, bk, Wv, bv, Wo, bo)
    res = run_bass_kernel_spmd(nc, in_maps, list(range(N_CORES)))
    out = np.empty((4, TQ, 1024), np.float32)
    for b in range(4):
        out[b] = res.results[2 * b]["out"] + res.results[2 * b + 1]["out"] + bo
    return out


# revision 11
# speedup vs baseline: 1.3656x; 1.3656x over previous
"""Trainium2 Bass kernel for nn_CrossAttention (B=4, T=2048, 1024 dims, 16 heads).

Sharding: 8 cores = 4 batches x 2 head-groups (8 heads each).
Per core (SPMD, same program, different data):
  phase 1: qT = Wq_g^T @ query_b^T + bq  (f32r, [512, 2048], head-dim on partitions)
           kT = Wk_g^T @ context_b^T + bk
           v  = context_b @ Wv_g + bv    ([2048, 512], with ones column per head)
  phase 2: per head h: scoresT = kT_h^T(stationary) x qT_h(moving) -> PSUM [tc, tq]
           probsT = exp(scoresT)  (ScalarE, no max subtraction: |scores| < 3)
           [attnT; denom] += [v_h, 1]^T x probsT  (PSUM accumulate over tc tiles)
           attnT_norm = attnT * (1/denom)  (broadcast via SWDGE DMA)
  phase 3: out_partial = attnT_norm^T @ Wo_g  -> DRAM
Host: out[b] = partial[2b] + partial[2b+1] + bo.   1/sqrt(D) folded into Wq/bq.
"""
import numpy as np
import concourse.bacc as bacc
import concourse.mybir as mybir
from concourse.tile import TileContext
from concourse.bass_utils import run_bass_kernel_spmd

N_CORES = 8
P = 128
F = 1024            # query/context feature dim
KF = F // P         # 8 f-tiles
HD = 512            # head-group hidden dim (8 heads x 64)
HT = HD // P        # 4 h-tiles
TQ = TC = 2048
NH = 8              # heads per core
D = 64
CH = 1024           # tq chunk for phase 2/3
NCH = TQ // CH      # 2
TCT = TC // P       # 16 tc tiles
F32 = mybir.dt.float32
F32R = mybir.dt.float32r
BF16 = mybir.dt.bfloat16
PH2_DT = BF16      # dtype for qT/kT/vv/probs (attention phase)

_NC_CACHE = None


def build_kernel(debug=False):
    nc = bacc.Bacc("TRN2", target_bir_lowering=False, debug=False, num_devices=N_CORES)

    qt_d = nc.dram_tensor("qt", [F, TQ], F32R, kind="ExternalInput")     # query[b].T
    ct_d = nc.dram_tensor("ct", [F, TC], F32R, kind="ExternalInput")     # context[b].T
    wq_d = nc.dram_tensor("wq", [F, HD], F32R, kind="ExternalInput")     # pre-scaled 1/8
    wk_d = nc.dram_tensor("wk", [F, HD], F32R, kind="ExternalInput")
    wv_d = nc.dram_tensor("wv", [F, HD], F32R, kind="ExternalInput")
    wo_d = nc.dram_tensor("wo", [HD, 1024], F32R, kind="ExternalInput")
    bq_d = nc.dram_tensor("bq", [HT, P], F32, kind="ExternalInput")      # pre-scaled 1/8
    bk_d = nc.dram_tensor("bk", [HT, P], F32, kind="ExternalInput")
    bv_d = nc.dram_tensor("bv", [1, HD], F32R, kind="ExternalInput")
    out_d = nc.dram_tensor("out", [TQ, 1024], F32, kind="ExternalOutput")
    scr_d = nc.dram_tensor("scr", [NCH, NH, CH], F32)  # denom-recip bounce for broadcast
    if debug:
        qT_dump = nc.dram_tensor("qT_dump", [P, HT, TQ], F32, kind="ExternalOutput")
        kT_dump = nc.dram_tensor("kT_dump", [P, HT, TC], F32, kind="ExternalOutput")
        vv_dump = nc.dram_tensor("vv_dump", [P, TCT, NH, D + 1], F32, kind="ExternalOutput")
        at_dump = nc.dram_tensor("at_dump", [NCH, P, HT, CH], F32, kind="ExternalOutput")
        pb_dump = nc.dram_tensor("pb_dump", [P, CH], F32, kind="ExternalOutput")
        pa_dump = nc.dram_tensor("pa_dump", [D + 1, CH], F32, kind="ExternalOutput")
        bc_dump = nc.dram_tensor("bc_dump", [D, CH], F32, kind="ExternalOutput")

    with TileContext(nc) as tc:
        with tc.tile_pool(name="consts", bufs=1) as consts, \
             tc.tile_pool(name="qkv", bufs=1) as qkv, \
             tc.tile_pool(name="ps_proj", bufs=2, space="PSUM") as ps_proj, \
             tc.tile_pool(name="ps_scores", bufs=2, space="PSUM") as ps_scores, \
             tc.tile_pool(name="ps_attn", bufs=1, space="PSUM") as ps_attn:

            ones = consts.tile([1, P], F32R)
            nc.vector.memset(ones.bitcast(F32), 1.0)
            bv_sb = consts.tile([1, HD], F32R)
            nc.sync.dma_start(out=bv_sb, in_=bv_d[:, :])
            bq_sb = consts.tile([P, HT], F32)
            nc.sync.dma_start(out=bq_sb, in_=bq_d.rearrange("m p -> p m"))
            bk_sb = consts.tile([P, HT], F32)
            nc.sync.dma_start(out=bk_sb, in_=bk_d.rearrange("m p -> p m"))

            qT = qkv.tile([P, HT, TQ], PH2_DT)        # [h-dim, tq]
            kT = qkv.tile([P, HT, TC], PH2_DT)        # [h-dim, tc]
            vv = qkv.tile([P, TCT, NH, D + 1], PH2_DT)  # per head: [v | ones]
            if PH2_DT == F32R:
                nc.vector.memset(vv.bitcast(F32)[:, :, :, D:D + 1], 1.0)
            else:
                nc.vector.memset(vv[:, :, :, D:D + 1], 1.0)

            # ---------------- phase 1: projections ----------------
            with tc.tile_pool(name="wts", bufs=1) as wts, \
                 tc.tile_pool(name="chunks", bufs=2) as chunks:
                wq = wts.tile([P, KF, HD], F32R)
                nc.sync.dma_start(out=wq, in_=wq_d.rearrange("(k p) h -> p k h", p=P))
                wk = wts.tile([P, KF, HD], F32R)
                nc.sync.dma_start(out=wk, in_=wk_d.rearrange("(k p) h -> p k h", p=P))
                wv = wts.tile([P, KF, HD], F32R)
                nc.sync.dma_start(out=wv, in_=wv_d.rearrange("(k p) h -> p k h", p=P))

                qt_r = qt_d.rearrange("(k p) t -> p k t", p=P)
                ct_r = ct_d.rearrange("(k p) t -> p k t", p=P)

                # qT = Wq^T @ queryT (+ bq per-partition on evacuation)
                for n in range(TQ // 512):
                    qc = chunks.tile([P, KF, 512], F32R, tag="chunk")
                    nc.sync.dma_start(out=qc, in_=qt_r[:, :, n * 512:(n + 1) * 512])
                    for m in range(HT):
                        ps = ps_proj.tile([P, 512], F32)
                        for k in range(KF):
                            nc.tensor.matmul(ps, wq[:, k, m * P:(m + 1) * P], qc[:, k, :],
                                             start=(k == 0), stop=(k == KF - 1))
                        nc.vector.tensor_scalar_add(
                            out=qT[:, m, n * 512:(n + 1) * 512], in0=ps,
                            scalar1=bq_sb[:, m:m + 1])
                # kT and v from shared contextT chunks
                for n in range(TC // 512):
                    cc = chunks.tile([P, KF, 512], F32R, tag="chunk")
                    nc.sync.dma_start(out=cc, in_=ct_r[:, :, n * 512:(n + 1) * 512])
                    for m in range(HT):
                        ps = ps_proj.tile([P, 512], F32)
                        for k in range(KF):
                            nc.tensor.matmul(ps, wk[:, k, m * P:(m + 1) * P], cc[:, k, :],
                                             start=(k == 0), stop=(k == KF - 1))
                        nc.vector.tensor_scalar_add(
                            out=kT[:, m, n * 512:(n + 1) * 512], in0=ps,
                            scalar1=bk_sb[:, m:m + 1])
                    for tl in range(4):
                        i = n * 4 + tl
                        ps = ps_proj.tile([P, 512], F32)
                        for k in range(KF):
                            nc.tensor.matmul(ps, cc[:, k, tl * P:(tl + 1) * P], wv[:, k, :],
                                             start=(k == 0), stop=False)
                        nc.tensor.matmul(ps, ones[0:1, 0:P], bv_sb[0:1, :],
                                         start=False, stop=True)
                        nc.vector.tensor_copy(
                            out=vv[:, i, :, 0:D],
                            in_=ps.rearrange("p (h d) -> p h d", h=NH))

            # ---------------- phase 2 + 3 ----------------
            with tc.tile_pool(name="wop", bufs=1) as wop, \
                 tc.tile_pool(name="probs", bufs=6) as probs, \
                 tc.tile_pool(name="attnsb", bufs=2) as attnsb, \
                 tc.tile_pool(name="norm", bufs=2) as norm, \
                 tc.tile_pool(name="outsb", bufs=2) as outsb:
                wo = wop.tile([P, HT, 1024], F32R)
                nc.sync.dma_start(out=wo, in_=wo_d.rearrange("(m p) o -> p m o", p=P))

                if debug and PH2_DT == F32R:
                    nc.sync.dma_start(out=qT_dump[:, :, :], in_=qT.bitcast(F32))
                    nc.sync.dma_start(out=kT_dump[:, :, :], in_=kT.bitcast(F32))
                    nc.sync.dma_start(out=vv_dump[:, :, :, :], in_=vv.bitcast(F32))
                for c in range(NCH):
                    # phase 2: attention for this tq chunk
                    at = attnsb.tile([P, HT, CH], F32R, tag="attnsb")
                    for h in range(NH):
                        ht, hp = h // 2, (h % 2) * D
                        pa = ps_attn.tile([D + 1, CH], F32)
                        for i in range(TCT):
                            ss = ps_scores.tile([P, CH], F32)
                            for n in range(CH // 512):
                                nc.tensor.matmul(
                                    ss[:, n * 512:(n + 1) * 512],
                                    kT[hp:hp + D, ht, i * P:(i + 1) * P],
                                    qT[hp:hp + D, ht,
                                       c * CH + n * 512:c * CH + (n + 1) * 512],
                                    start=True, stop=True)
                            pb = probs.tile([P, CH], PH2_DT)
                            nc.scalar.activation(pb, ss, mybir.ActivationFunctionType.Exp)
                            if debug and c == 0 and h == 0 and i == 0:
                                nc.sync.dma_start(out=pb_dump[:, :], in_=pb.bitcast(F32))
                            for n in range(CH // 512):
                                nc.tensor.matmul(
                                    pa[:, n * 512:(n + 1) * 512],
                                    vv[:, i, h, :],
                                    pb[:, n * 512:(n + 1) * 512],
                                    start=(i == 0), stop=(i == TCT - 1))
                        if debug and c == 0 and h == 0:
                            pa_sb = norm.tile([D + 1, CH], F32, tag="pa_dbg")
                            nc.vector.tensor_copy(out=pa_sb, in_=pa)
                            nc.sync.dma_start(out=pa_dump[:, :], in_=pa_sb)
                        # normalize: recip of denom row, broadcast, multiply
                        rc = norm.tile([D + 1, CH], F32, tag="rc")
                        nc.vector.reciprocal(out=rc[D:D + 1, :], in_=pa[D:D + 1, :])
                        bc = norm.tile([D, CH], F32, tag="bc")
                        nc.sync.dma_start(out=scr_d[c, h:h + 1, :], in_=rc[D:D + 1, :])
                        nc.gpsimd.dma_start(
                            out=bc, in_=scr_d[c, h:h + 1, :].to_broadcast([D, CH]))
                        if debug and c == 0 and h == 0:
                            nc.sync.dma_start(out=bc_dump[:, :], in_=bc)
                        if h % 2 == 0:
                            nc.vector.tensor_mul(out=at[0:D, ht, :], in0=pa[0:D, :], in1=bc)
                        else:
                            tmp = norm.tile([D, CH], F32R, tag="tmp")
                            nc.vector.tensor_mul(out=tmp, in0=pa[0:D, :], in1=bc)
                            nc.gpsimd.dma_start(out=at[D:P, ht, :], in_=tmp)
                    if debug:
                        nc.sync.dma_start(out=at_dump[c], in_=at.bitcast(F32))
                    # phase 3: output projection for this chunk
                    for t in range(CH // P):
                        ot = outsb.tile([P, 1024], F32)
                        for o in range(2):
                            po = ps_proj.tile([P, 512], F32, tag="ps")
                            for m in range(HT):
                                nc.tensor.matmul(po, at[:, m, t * P:(t + 1) * P],
                                                 wo[:, m, o * 512:(o + 1) * 512],
                                                 start=(m == 0), stop=(m == HT - 1))
                            nc.vector.tensor_copy(out=ot[:, o * 512:(o + 1) * 512], in_=po)
                        nc.sync.dma_start(
                            out=out_d[c * CH + t * P:c * CH + (t + 1) * P, :], in_=ot)

    nc.compile()
    return nc


def make_in_maps(query, context, Wq, bq, Wk, bk, Wv, bv, Wo, bo):
    query = np.asarray(query, np.float32)
    context = np.asarray(context, np.float32)
    Wq = np.asarray(Wq, np.float32); bq = np.asarray(bq, np.float32)
    Wk = np.asarray(Wk, np.float32); bk = np.asarray(bk, np.float32)
    Wv = np.asarray(Wv, np.float32); bv = np.asarray(bv, np.float32)
    Wo = np.asarray(Wo, np.float32)

    in_maps = []
    for c in range(N_CORES):
        b, g = c // 2, c % 2
        sl = slice(g * HD, (g + 1) * HD)
        in_maps.append({
            "qt": np.ascontiguousarray(query[b].T),
            "ct": np.ascontiguousarray(context[b].T),
            "wq": np.ascontiguousarray(Wq[:, sl] * 0.125),
            "wk": np.ascontiguousarray(Wk[:, sl]),
            "wv": np.ascontiguousarray(Wv[:, sl]),
            "wo": np.ascontiguousarray(Wo[sl, :]),
            "bq": np.ascontiguousarray((bq[sl] * 0.125).reshape(HT, P)),
            "bk": np.ascontiguousarray(bk[sl].reshape(HT, P)),
            "bv": np.ascontiguousarray(bv[sl].reshape(1, HD)),
        })
    return in_maps


def kernel(query, context, Wq, bq, Wk, bk, Wv, bv, Wo, bo):
    global _NC_CACHE
    if _NC_CACHE is None:
        _NC_CACHE = build_kernel()
    nc = _NC_CACHE
    bo = np.asarray(bo, np.float32)

    in_maps = make_in_maps(query, context, Wq, bq, Wk, bk, Wv, bv, Wo, bo)
    res = run_bass_kernel_spmd(nc, in_maps, list(range(N_CORES)))
    out = np.empty((4, TQ, 1024), np.float32)
    for b in range(4):
        out[b] = res.results[2 * b]["out"] + res.results[2 * b + 1]["out"] + bo
    return out


# revision 14
# speedup vs baseline: 1.7897x; 1.3106x over previous
"""Trainium2 Bass kernel for nn_CrossAttention (B=4, T=2048, 1024 dims, 16 heads).

Sharding: 8 cores = 4 batches x 2 head-groups (8 heads each).
Per core (SPMD, same program, different data):
  phase 1: qT = Wq_g^T @ query_b^T + bq  (f32r, [512, 2048], head-dim on partitions)
           kT = Wk_g^T @ context_b^T + bk
           v  = context_b @ Wv_g + bv    ([2048, 512], with ones column per head)
  phase 2: per head h: scoresT = kT_h^T(stationary) x qT_h(moving) -> PSUM [tc, tq]
           probsT = exp(scoresT)  (ScalarE, no max subtraction: |scores| < 3)
           [attnT; denom] += [v_h, 1]^T x probsT  (PSUM accumulate over tc tiles)
           attnT_norm = attnT * (1/denom)  (broadcast via SWDGE DMA)
  phase 3: out_partial = attnT_norm^T @ Wo_g  -> DRAM
Host: out[b] = partial[2b] + partial[2b+1] + bo.   1/sqrt(D) folded into Wq/bq.
"""
import numpy as np
import concourse.bacc as bacc
import concourse.mybir as mybir
from concourse.tile import TileContext
from concourse.bass_utils import run_bass_kernel_spmd

N_CORES = 8
P = 128
F = 1024            # query/context feature dim
KF = F // P         # 8 f-tiles
HD = 512            # head-group hidden dim (8 heads x 64)
HT = HD // P        # 4 h-tiles
TQ = TC = 2048
NH = 8              # heads per core
D = 64
CH = 1024           # tq chunk for phase 2/3
NCH = TQ // CH      # 2
TCT = TC // P       # 16 tc tiles
F32 = mybir.dt.float32
F32R = mybir.dt.float32r
BF16 = mybir.dt.bfloat16
PH2_DT = BF16      # dtype for qT/kT/vv/probs (attention phase)

_NC_CACHE = None


def build_kernel(debug=False):
    nc = bacc.Bacc("TRN2", target_bir_lowering=False, debug=False, num_devices=N_CORES)

    qt_d = nc.dram_tensor("qt", [F, TQ], F32R, kind="ExternalInput")     # query[b].T
    ct_d = nc.dram_tensor("ct", [F, TC], F32R, kind="ExternalInput")     # context[b].T
    wq_d = nc.dram_tensor("wq", [F, HD], F32R, kind="ExternalInput")     # pre-scaled 1/8
    wk_d = nc.dram_tensor("wk", [F, HD], F32R, kind="ExternalInput")
    wv_d = nc.dram_tensor("wv", [F, HD], F32R, kind="ExternalInput")
    wo_d = nc.dram_tensor("wo", [HD, 1024], F32R, kind="ExternalInput")
    bq_d = nc.dram_tensor("bq", [HT, P], F32, kind="ExternalInput")      # pre-scaled 1/8
    bk_d = nc.dram_tensor("bk", [HT, P], F32, kind="ExternalInput")
    bv_d = nc.dram_tensor("bv", [1, HD], F32R, kind="ExternalInput")
    out_d = nc.dram_tensor("out", [TQ, 1024], F32, kind="ExternalOutput")
    scr_d = nc.dram_tensor("scr", [NCH, NH, CH], F32)  # denom-recip bounce for broadcast
    if debug:
        qT_dump = nc.dram_tensor("qT_dump", [P, HT, TQ], F32, kind="ExternalOutput")
        kT_dump = nc.dram_tensor("kT_dump", [P, HT, TC], F32, kind="ExternalOutput")
        vv_dump = nc.dram_tensor("vv_dump", [P, TCT, NH, D + 1], F32, kind="ExternalOutput")
        at_dump = nc.dram_tensor("at_dump", [NCH, P, HT, CH], F32, kind="ExternalOutput")
        pb_dump = nc.dram_tensor("pb_dump", [P, CH], F32, kind="ExternalOutput")
        pa_dump = nc.dram_tensor("pa_dump", [D + 1, CH], F32, kind="ExternalOutput")
        bc_dump = nc.dram_tensor("bc_dump", [D, CH], F32, kind="ExternalOutput")

    with TileContext(nc) as tc:
        with tc.tile_pool(name="consts", bufs=1) as consts, \
             tc.tile_pool(name="qkv", bufs=1) as qkv, \
             tc.tile_pool(name="ps_proj", bufs=2, space="PSUM") as ps_proj, \
             tc.tile_pool(name="ps_scores", bufs=2, space="PSUM") as ps_scores, \
             tc.tile_pool(name="ps_attn", bufs=1, space="PSUM") as ps_attn:

            ones = consts.tile([1, P], F32R)
            nc.vector.memset(ones.bitcast(F32), 1.0)
            bv_sb = consts.tile([1, HD], F32R)
            nc.sync.dma_start(out=bv_sb, in_=bv_d[:, :])
            bq_sb = consts.tile([P, HT], F32)
            nc.sync.dma_start(out=bq_sb, in_=bq_d.rearrange("m p -> p m"))
            bk_sb = consts.tile([P, HT], F32)
            nc.sync.dma_start(out=bk_sb, in_=bk_d.rearrange("m p -> p m"))

            qT = qkv.tile([P, HT, TQ], PH2_DT)        # [h-dim, tq]
            kT = qkv.tile([P, HT, TC], PH2_DT)        # [h-dim, tc]
            vv = qkv.tile([P, TCT, NH, D + 1], PH2_DT)  # per head: [v | ones]
            if PH2_DT == F32R:
                nc.vector.memset(vv.bitcast(F32)[:, :, :, D:D + 1], 1.0)
            else:
                nc.vector.memset(vv[:, :, :, D:D + 1], 1.0)

            # ---------------- phase 1: projections ----------------
            with tc.tile_pool(name="wts", bufs=1) as wts, \
                 tc.tile_pool(name="chunks", bufs=2) as chunks:
                wq = wts.tile([P, KF, HD], F32R)
                nc.sync.dma_start(out=wq, in_=wq_d.rearrange("(k p) h -> p k h", p=P))
                wk = wts.tile([P, KF, HD], F32R)
                nc.sync.dma_start(out=wk, in_=wk_d.rearrange("(k p) h -> p k h", p=P))
                wv = wts.tile([P, KF, HD], F32R)
                nc.sync.dma_start(out=wv, in_=wv_d.rearrange("(k p) h -> p k h", p=P))

                qt_r = qt_d.rearrange("(k p) t -> p k t", p=P)
                ct_r = ct_d.rearrange("(k p) t -> p k t", p=P)

                # qT = Wq^T @ queryT (+ bq per-partition on evacuation)
                for n in range(TQ // 512):
                    qc = chunks.tile([P, KF, 512], F32R, tag="chunk")
                    nc.sync.dma_start(out=qc, in_=qt_r[:, :, n * 512:(n + 1) * 512])
                    for m in range(HT):
                        ps = ps_proj.tile([P, 512], F32)
                        for k in range(KF):
                            nc.tensor.matmul(ps, wq[:, k, m * P:(m + 1) * P], qc[:, k, :],
                                             start=(k == 0), stop=(k == KF - 1))
                        nc.vector.tensor_scalar_add(
                            out=qT[:, m, n * 512:(n + 1) * 512], in0=ps,
                            scalar1=bq_sb[:, m:m + 1])
                # kT and v from shared contextT chunks
                for n in range(TC // 512):
                    cc = chunks.tile([P, KF, 512], F32R, tag="chunk")
                    nc.sync.dma_start(out=cc, in_=ct_r[:, :, n * 512:(n + 1) * 512])
                    for m in range(HT):
                        ps = ps_proj.tile([P, 512], F32)
                        for k in range(KF):
                            nc.tensor.matmul(ps, wk[:, k, m * P:(m + 1) * P], cc[:, k, :],
                                             start=(k == 0), stop=(k == KF - 1))
                        nc.vector.tensor_scalar_add(
                            out=kT[:, m, n * 512:(n + 1) * 512], in0=ps,
                            scalar1=bk_sb[:, m:m + 1])
                    for tl in range(4):
                        i = n * 4 + tl
                        ps = ps_proj.tile([P, 512], F32)
                        for k in range(KF):
                            nc.tensor.matmul(ps, cc[:, k, tl * P:(tl + 1) * P], wv[:, k, :],
                                             start=(k == 0), stop=False)
                        nc.tensor.matmul(ps, ones[0:1, 0:P], bv_sb[0:1, :],
                                         start=False, stop=True)
                        nc.vector.tensor_copy(
                            out=vv[:, i, :, 0:D],
                            in_=ps.rearrange("p (h d) -> p h d", h=NH))

            # ---------------- phase 2 + 3 ----------------
            with tc.tile_pool(name="wop", bufs=1) as wop, \
                 tc.tile_pool(name="probs", bufs=6) as probs, \
                 tc.tile_pool(name="attnsb", bufs=2) as attnsb, \
                 tc.tile_pool(name="norm", bufs=2) as norm, \
                 tc.tile_pool(name="outsb", bufs=2) as outsb:
                wo = wop.tile([P, HT, 1024], F32R)
                nc.sync.dma_start(out=wo, in_=wo_d.rearrange("(m p) o -> p m o", p=P))

                if debug and PH2_DT == F32R:
                    nc.sync.dma_start(out=qT_dump[:, :, :], in_=qT.bitcast(F32))
                    nc.sync.dma_start(out=kT_dump[:, :, :], in_=kT.bitcast(F32))
                    nc.sync.dma_start(out=vv_dump[:, :, :, :], in_=vv.bitcast(F32))
                for c in range(NCH):
                    # phase 2: attention for this tq chunk
                    at = attnsb.tile([P, HT, CH], F32R, tag="attnsb")
                    for h in range(NH):
                        ht, hp = h // 2, (h % 2) * D
                        pa = ps_attn.tile([D + 1, CH], F32)
                        for i in range(TCT):
                            ss = ps_scores.tile([P, CH], F32)
                            for n in range(CH // 512):
                                nc.tensor.matmul(
                                    ss[:, n * 512:(n + 1) * 512],
                                    kT[hp:hp + D, ht, i * P:(i + 1) * P],
                                    qT[hp:hp + D, ht,
                                       c * CH + n * 512:c * CH + (n + 1) * 512],
                                    start=True, stop=True)
                            pb = probs.tile([P, CH], PH2_DT)
                            nc.scalar.activation(pb, ss, mybir.ActivationFunctionType.Exp)
                            if debug and c == 0 and h == 0 and i == 0:
                                pbf = probs.tile([P, CH], F32, tag="pb_dbg")
                                nc.vector.tensor_copy(out=pbf, in_=pb)
                                nc.sync.dma_start(out=pb_dump[:, :], in_=pbf)
                            for n in range(CH // 512):
                                nc.tensor.matmul(
                                    pa[:, n * 512:(n + 1) * 512],
                                    vv[:, i, h, :],
                                    pb[:, n * 512:(n + 1) * 512],
                                    start=(i == 0), stop=(i == TCT - 1))
                        # normalize: evacuate PSUM promptly (frees pa for next head),
                        # approx-recip of denom row, broadcast via DRAM bounce, multiply
                        pa_sb = norm.tile([D + 1, CH], F32, tag="pasb")
                        nc.vector.tensor_copy(out=pa_sb, in_=pa)
                        if debug and c == 0 and h == 0:
                            nc.sync.dma_start(out=pa_dump[:, :], in_=pa_sb)
                        nc.sync.dma_start(out=scr_d[c, h:h + 1, :], in_=pa_sb[D:D + 1, :])
                        dn = norm.tile([D, CH], F32, tag="dn")
                        nc.gpsimd.dma_start(
                            out=dn, in_=scr_d[c, h:h + 1, :].to_broadcast([D, CH]))
                        bc = norm.tile([D, CH], F32, tag="bc")
                        nc.vector.reciprocal_approx_fast(out=bc, in_=dn)
                        if debug and c == 0 and h == 0:
                            nc.sync.dma_start(out=bc_dump[:, :], in_=bc)
                        if h % 2 == 0:
                            nc.vector.tensor_mul(out=at[0:D, ht, :], in0=pa_sb[0:D, :], in1=bc)
                        else:
                            tmp = norm.tile([D, CH], F32R, tag="tmp")
                            nc.vector.tensor_mul(out=tmp, in0=pa_sb[0:D, :], in1=bc)
                            nc.gpsimd.dma_start(out=at[D:P, ht, :], in_=tmp)
                    if debug:
                        atf = attnsb.tile([P, HT, CH], F32, tag="at_dbg")
                        nc.vector.tensor_copy(out=atf, in_=at)
                        nc.sync.dma_start(out=at_dump[c], in_=atf)
                    # phase 3: output projection for this chunk
                    for t in range(CH // P):
                        ot = outsb.tile([P, 1024], F32)
                        for o in range(2):
                            po = ps_proj.tile([P, 512], F32, tag="ps")
                            for m in range(HT):
                                nc.tensor.matmul(po, at[:, m, t * P:(t + 1) * P],
                                                 wo[:, m, o * 512:(o + 1) * 512],
                                                 start=(m == 0), stop=(m == HT - 1))
                            nc.vector.tensor_copy(out=ot[:, o * 512:(o + 1) * 512], in_=po)
                        nc.sync.dma_start(
                            out=out_d[c * CH + t * P:c * CH + (t + 1) * P, :], in_=ot)

    nc.compile()
    return nc


def make_in_maps(query, context, Wq, bq, Wk, bk, Wv, bv, Wo, bo):
    query = np.asarray(query, np.float32)
    context = np.asarray(context, np.float32)
    Wq = np.asarray(Wq, np.float32); bq = np.asarray(bq, np.float32)
    Wk = np.asarray(Wk, np.float32); bk = np.asarray(bk, np.float32)
    Wv = np.asarray(Wv, np.float32); bv = np.asarray(bv, np.float32)
    Wo = np.asarray(Wo, np.float32)

    in_maps = []
    for c in range(N_CORES):
        b, g = c // 2, c % 2
        sl = slice(g * HD, (g + 1) * HD)
        in_maps.append({
            "qt": np.ascontiguousarray(query[b].T),
            "ct": np.ascontiguousarray(context[b].T),
            "wq": np.ascontiguousarray(Wq[:, sl] * 0.125),
            "wk": np.ascontiguousarray(Wk[:, sl]),
            "wv": np.ascontiguousarray(Wv[:, sl]),
            "wo": np.ascontiguousarray(Wo[sl, :]),
            "bq": np.ascontiguousarray((bq[sl] * 0.125).reshape(HT, P)),
            "bk": np.ascontiguousarray(bk[sl].reshape(HT, P)),
            "bv": np.ascontiguousarray(bv[sl].reshape(1, HD)),
        })
    return in_maps


def kernel(query, context, Wq, bq, Wk, bk, Wv, bv, Wo, bo):
    global _NC_CACHE
    if _NC_CACHE is None:
        _NC_CACHE = build_kernel()
    nc = _NC_CACHE
    bo = np.asarray(bo, np.float32)

    in_maps = make_in_maps(query, context, Wq, bq, Wk, bk, Wv, bv, Wo, bo)
    res = run_bass_kernel_spmd(nc, in_maps, list(range(N_CORES)))
    out = np.empty((4, TQ, 1024), np.float32)
    for b in range(4):
        out[b] = res.results[2 * b]["out"] + res.results[2 * b + 1]["out"] + bo
    return out


# revision 15
# speedup vs baseline: 1.9688x; 1.1001x over previous
"""Trainium2 Bass kernel for nn_CrossAttention (B=4, T=2048, 1024 dims, 16 heads).

Sharding: 8 cores = 4 batches x 2 head-groups (8 heads each).
Per core (SPMD, same program, different data):
  phase 1: qT = Wq_g^T @ query_b^T + bq  (f32r, [512, 2048], head-dim on partitions)
           kT = Wk_g^T @ context_b^T + bk
           v  = context_b @ Wv_g + bv    ([2048, 512], with ones column per head)
  phase 2: per head h: scoresT = kT_h^T(stationary) x qT_h(moving) -> PSUM [tc, tq]
           probsT = exp(scoresT)  (ScalarE, no max subtraction: |scores| < 3)
           [attnT; denom] += [v_h, 1]^T x probsT  (PSUM accumulate over tc tiles)
           attnT_norm = attnT * (1/denom)  (broadcast via SWDGE DMA)
  phase 3: out_partial = attnT_norm^T @ Wo_g  -> DRAM
Host: out[b] = partial[2b] + partial[2b+1] + bo.   1/sqrt(D) folded into Wq/bq.
"""
import numpy as np
import concourse.bacc as bacc
import concourse.mybir as mybir
from concourse.tile import TileContext
from concourse.bass_utils import run_bass_kernel_spmd

N_CORES = 8
P = 128
F = 1024            # query/context feature dim
KF = F // P         # 8 f-tiles
HD = 512            # head-group hidden dim (8 heads x 64)
HT = HD // P        # 4 h-tiles
TQ = TC = 2048
NH = 8              # heads per core
D = 64
CH = 1024           # tq chunk for phase 2/3
NCH = TQ // CH      # 2
TCT = TC // P       # 16 tc tiles
F32 = mybir.dt.float32
F32R = mybir.dt.float32r
BF16 = mybir.dt.bfloat16
PH2_DT = BF16      # dtype for qT/kT/vv/probs (attention phase)
PH1_DT = BF16      # dtype for inputs/weights (projection phases); host pre-casts

_NC_CACHE = None


def build_kernel(debug=False):
    nc = bacc.Bacc("TRN2", target_bir_lowering=False, debug=False, num_devices=N_CORES)

    qt_d = nc.dram_tensor("qt", [F, TQ], PH1_DT, kind="ExternalInput")     # query[b].T
    ct_d = nc.dram_tensor("ct", [F, TC], PH1_DT, kind="ExternalInput")     # context[b].T
    wq_d = nc.dram_tensor("wq", [F, HD], PH1_DT, kind="ExternalInput")     # pre-scaled 1/8
    wk_d = nc.dram_tensor("wk", [F, HD], PH1_DT, kind="ExternalInput")
    wv_d = nc.dram_tensor("wv", [F, HD], PH1_DT, kind="ExternalInput")
    wo_d = nc.dram_tensor("wo", [HD, 1024], PH1_DT, kind="ExternalInput")
    bq_d = nc.dram_tensor("bq", [HT, P], F32, kind="ExternalInput")      # pre-scaled 1/8
    bk_d = nc.dram_tensor("bk", [HT, P], F32, kind="ExternalInput")
    bv_d = nc.dram_tensor("bv", [1, HD], PH1_DT, kind="ExternalInput")
    out_d = nc.dram_tensor("out", [TQ, 1024], F32, kind="ExternalOutput")
    scr_d = nc.dram_tensor("scr", [NCH, NH, CH], F32)  # denom-recip bounce for broadcast
    if debug:
        qT_dump = nc.dram_tensor("qT_dump", [P, HT, TQ], F32, kind="ExternalOutput")
        kT_dump = nc.dram_tensor("kT_dump", [P, HT, TC], F32, kind="ExternalOutput")
        vv_dump = nc.dram_tensor("vv_dump", [P, TCT, NH, D + 1], F32, kind="ExternalOutput")
        at_dump = nc.dram_tensor("at_dump", [NCH, P, HT, CH], F32, kind="ExternalOutput")
        pb_dump = nc.dram_tensor("pb_dump", [P, CH], F32, kind="ExternalOutput")
        pa_dump = nc.dram_tensor("pa_dump", [D + 1, CH], F32, kind="ExternalOutput")
        bc_dump = nc.dram_tensor("bc_dump", [D, CH], F32, kind="ExternalOutput")

    with TileContext(nc) as tc:
        with tc.tile_pool(name="consts", bufs=1) as consts, \
             tc.tile_pool(name="qkv", bufs=1) as qkv, \
             tc.tile_pool(name="ps_proj", bufs=2, space="PSUM") as ps_proj, \
             tc.tile_pool(name="ps_scores", bufs=2, space="PSUM") as ps_scores, \
             tc.tile_pool(name="ps_attn", bufs=1, space="PSUM") as ps_attn:

            ones = consts.tile([1, P], PH1_DT)
            if PH1_DT == F32R:
                nc.vector.memset(ones.bitcast(F32), 1.0)
            else:
                nc.vector.memset(ones, 1.0)
            bv_sb = consts.tile([1, HD], PH1_DT)
            nc.sync.dma_start(out=bv_sb, in_=bv_d[:, :])
            bq_sb = consts.tile([P, HT], F32)
            nc.sync.dma_start(out=bq_sb, in_=bq_d.rearrange("m p -> p m"))
            bk_sb = consts.tile([P, HT], F32)
            nc.sync.dma_start(out=bk_sb, in_=bk_d.rearrange("m p -> p m"))

            qT = qkv.tile([P, HT, TQ], PH2_DT)        # [h-dim, tq]
            kT = qkv.tile([P, HT, TC], PH2_DT)        # [h-dim, tc]
            vv = qkv.tile([P, TCT, NH, D + 1], PH2_DT)  # per head: [v | ones]
            if PH2_DT == F32R:
                nc.vector.memset(vv.bitcast(F32)[:, :, :, D:D + 1], 1.0)
            else:
                nc.vector.memset(vv[:, :, :, D:D + 1], 1.0)

            # ---------------- phase 1: projections ----------------
            with tc.tile_pool(name="wts", bufs=1) as wts, \
                 tc.tile_pool(name="chunks", bufs=2) as chunks:
                wq = wts.tile([P, KF, HD], PH1_DT)
                nc.sync.dma_start(out=wq, in_=wq_d.rearrange("(k p) h -> p k h", p=P))
                wk = wts.tile([P, KF, HD], PH1_DT)
                nc.sync.dma_start(out=wk, in_=wk_d.rearrange("(k p) h -> p k h", p=P))
                wv = wts.tile([P, KF, HD], PH1_DT)
                nc.sync.dma_start(out=wv, in_=wv_d.rearrange("(k p) h -> p k h", p=P))

                qt_r = qt_d.rearrange("(k p) t -> p k t", p=P)
                ct_r = ct_d.rearrange("(k p) t -> p k t", p=P)

                # qT = Wq^T @ queryT (+ bq per-partition on evacuation)
                for n in range(TQ // 512):
                    qc = chunks.tile([P, KF, 512], PH1_DT, tag="chunk")
                    nc.sync.dma_start(out=qc, in_=qt_r[:, :, n * 512:(n + 1) * 512])
                    for m in range(HT):
                        ps = ps_proj.tile([P, 512], F32)
                        for k in range(KF):
                            nc.tensor.matmul(ps, wq[:, k, m * P:(m + 1) * P], qc[:, k, :],
                                             start=(k == 0), stop=(k == KF - 1))
                        nc.vector.tensor_scalar_add(
                            out=qT[:, m, n * 512:(n + 1) * 512], in0=ps,
                            scalar1=bq_sb[:, m:m + 1])
                # kT and v from shared contextT chunks
                for n in range(TC // 512):
                    cc = chunks.tile([P, KF, 512], PH1_DT, tag="chunk")
                    nc.sync.dma_start(out=cc, in_=ct_r[:, :, n * 512:(n + 1) * 512])
                    for m in range(HT):
                        ps = ps_proj.tile([P, 512], F32)
                        for k in range(KF):
                            nc.tensor.matmul(ps, wk[:, k, m * P:(m + 1) * P], cc[:, k, :],
                                             start=(k == 0), stop=(k == KF - 1))
                        nc.vector.tensor_scalar_add(
                            out=kT[:, m, n * 512:(n + 1) * 512], in0=ps,
                            scalar1=bk_sb[:, m:m + 1])
                    for tl in range(4):
                        i = n * 4 + tl
                        ps = ps_proj.tile([P, 512], F32)
                        for k in range(KF):
                            nc.tensor.matmul(ps, cc[:, k, tl * P:(tl + 1) * P], wv[:, k, :],
                                             start=(k == 0), stop=False)
                        nc.tensor.matmul(ps, ones[0:1, 0:P], bv_sb[0:1, :],
                                         start=False, stop=True)
                        nc.vector.tensor_copy(
                            out=vv[:, i, :, 0:D],
                            in_=ps.rearrange("p (h d) -> p h d", h=NH))

            # ---------------- phase 2 + 3 ----------------
            with tc.tile_pool(name="wop", bufs=1) as wop, \
                 tc.tile_pool(name="probs", bufs=6) as probs, \
                 tc.tile_pool(name="attnsb", bufs=2) as attnsb, \
                 tc.tile_pool(name="norm", bufs=2) as norm, \
                 tc.tile_pool(name="outsb", bufs=2) as outsb:
                wo = wop.tile([P, HT, 1024], PH1_DT)
                nc.sync.dma_start(out=wo, in_=wo_d.rearrange("(m p) o -> p m o", p=P))

                if debug and PH2_DT == F32R:
                    nc.sync.dma_start(out=qT_dump[:, :, :], in_=qT.bitcast(F32))
                    nc.sync.dma_start(out=kT_dump[:, :, :], in_=kT.bitcast(F32))
                    nc.sync.dma_start(out=vv_dump[:, :, :, :], in_=vv.bitcast(F32))
                for c in range(NCH):
                    # phase 2: attention for this tq chunk
                    at = attnsb.tile([P, HT, CH], PH1_DT, tag="attnsb")
                    for h in range(NH):
                        ht, hp = h // 2, (h % 2) * D
                        pa = ps_attn.tile([D + 1, CH], F32)
                        for i in range(TCT):
                            ss = ps_scores.tile([P, CH], F32)
                            for n in range(CH // 512):
                                nc.tensor.matmul(
                                    ss[:, n * 512:(n + 1) * 512],
                                    kT[hp:hp + D, ht, i * P:(i + 1) * P],
                                    qT[hp:hp + D, ht,
                                       c * CH + n * 512:c * CH + (n + 1) * 512],
                                    start=True, stop=True)
                            pb = probs.tile([P, CH], PH2_DT)
                            nc.scalar.activation(pb, ss, mybir.ActivationFunctionType.Exp)
                            if debug and c == 0 and h == 0 and i == 0:
                                pbf = probs.tile([P, CH], F32, tag="pb_dbg")
                                nc.vector.tensor_copy(out=pbf, in_=pb)
                                nc.sync.dma_start(out=pb_dump[:, :], in_=pbf)
                            for n in range(CH // 512):
                                nc.tensor.matmul(
                                    pa[:, n * 512:(n + 1) * 512],
                                    vv[:, i, h, :],
                                    pb[:, n * 512:(n + 1) * 512],
                                    start=(i == 0), stop=(i == TCT - 1))
                        # normalize: evacuate PSUM promptly (frees pa for next head),
                        # approx-recip of denom row, broadcast via DRAM bounce, multiply
                        pa_sb = norm.tile([D + 1, CH], F32, tag="pasb")
                        nc.vector.tensor_copy(out=pa_sb, in_=pa)
                        if debug and c == 0 and h == 0:
                            nc.sync.dma_start(out=pa_dump[:, :], in_=pa_sb)
                        nc.sync.dma_start(out=scr_d[c, h:h + 1, :], in_=pa_sb[D:D + 1, :])
                        dn = norm.tile([D, CH], F32, tag="dn")
                        nc.gpsimd.dma_start(
                            out=dn, in_=scr_d[c, h:h + 1, :].to_broadcast([D, CH]))
                        bc = norm.tile([D, CH], F32, tag="bc")
                        nc.vector.reciprocal_approx_fast(out=bc, in_=dn)
                        if debug and c == 0 and h == 0:
                            nc.sync.dma_start(out=bc_dump[:, :], in_=bc)
                        if h % 2 == 0:
                            nc.vector.tensor_mul(out=at[0:D, ht, :], in0=pa_sb[0:D, :], in1=bc)
                        else:
                            tmp = norm.tile([D, CH], PH1_DT, tag="tmp")
                            nc.vector.tensor_mul(out=tmp, in0=pa_sb[0:D, :], in1=bc)
                            nc.gpsimd.dma_start(out=at[D:P, ht, :], in_=tmp)
                    if debug:
                        atf = attnsb.tile([P, HT, CH], F32, tag="at_dbg")
                        nc.vector.tensor_copy(out=atf, in_=at)
                        nc.sync.dma_start(out=at_dump[c], in_=atf)
                    # phase 3: output projection for this chunk
                    for t in range(CH // P):
                        ot = outsb.tile([P, 1024], F32)
                        for o in range(2):
                            po = ps_proj.tile([P, 512], F32, tag="ps")
                            for m in range(HT):
                                nc.tensor.matmul(po, at[:, m, t * P:(t + 1) * P],
                                                 wo[:, m, o * 512:(o + 1) * 512],
                                                 start=(m == 0), stop=(m == HT - 1))
                            nc.vector.tensor_copy(out=ot[:, o * 512:(o + 1) * 512], in_=po)
                        nc.sync.dma_start(
                            out=out_d[c * CH + t * P:c * CH + (t + 1) * P, :], in_=ot)

    nc.compile()
    return nc


def make_in_maps(query, context, Wq, bq, Wk, bk, Wv, bv, Wo, bo):
    import ml_dtypes
    cast1 = (lambda a: np.asarray(a, np.float32)) if PH1_DT == F32R \
        else (lambda a: np.asarray(a, np.float32).astype(ml_dtypes.bfloat16))
    query = np.asarray(query, np.float32)
    context = np.asarray(context, np.float32)
    Wq = np.asarray(Wq, np.float32); bq = np.asarray(bq, np.float32)
    Wk = np.asarray(Wk, np.float32); bk = np.asarray(bk, np.float32)
    Wv = np.asarray(Wv, np.float32); bv = np.asarray(bv, np.float32)
    Wo = np.asarray(Wo, np.float32)

    in_maps = []
    for c in range(N_CORES):
        b, g = c // 2, c % 2
        sl = slice(g * HD, (g + 1) * HD)
        in_maps.append({
            "qt": cast1(np.ascontiguousarray(query[b].T)),
            "ct": cast1(np.ascontiguousarray(context[b].T)),
            "wq": cast1(np.ascontiguousarray(Wq[:, sl] * 0.125)),
            "wk": cast1(np.ascontiguousarray(Wk[:, sl])),
            "wv": cast1(np.ascontiguousarray(Wv[:, sl])),
            "wo": cast1(np.ascontiguousarray(Wo[sl, :])),
            "bq": np.ascontiguousarray((bq[sl] * 0.125).reshape(HT, P)),
            "bk": np.ascontiguousarray(bk[sl].reshape(HT, P)),
            "bv": cast1(bv[sl].reshape(1, HD)),
        })
    return in_maps


def kernel(query, context, Wq, bq, Wk, bk, Wv, bv, Wo, bo):
    global _NC_CACHE
    if _NC_CACHE is None:
        _NC_CACHE = build_kernel()
    nc = _NC_CACHE
    bo = np.asarray(bo, np.float32)

    in_maps = make_in_maps(query, context, Wq, bq, Wk, bk, Wv, bv, Wo, bo)
    res = run_bass_kernel_spmd(nc, in_maps, list(range(N_CORES)))
    out = np.empty((4, TQ, 1024), np.float32)
    for b in range(4):
        out[b] = res.results[2 * b]["out"] + res.results[2 * b + 1]["out"] + bo
    return out


# revision 16
# speedup vs baseline: 1.9942x; 1.0129x over previous
"""Trainium2 Bass kernel for nn_CrossAttention (B=4, T=2048, 1024 dims, 16 heads).

Sharding: 8 cores = 4 batches x 2 head-groups (8 heads each). Host sums the two
head-group partial outputs per batch and adds bo; 1/sqrt(D) folded into Wq/bq.

Per core, bf16 matmuls (PSUM accumulation fp32):
  phase 1: kT = Wk_g^T @ context_b^T + bk   ([512, 2048], head-dim on partitions)
           v  = context_b @ Wv_g + bv       ([2048, 512], ones column per head)
           qT = Wq_g^T @ query_b^T + bq     (first tq half; second half interleaved)
  phase 2: per head: scoresT = kT_h(stationary) x qT_h(moving) -> PSUM [tc, tq]
           probsT = exp(scoresT)            (ScalarE; no max subtraction, |s| < 3)
           [attnT; denom] += [v_h | 1]^T x probsT   (PSUM accum over tc tiles)
           attnT_norm = attnT * recip(denom)        (broadcast via DRAM bounce)
  phase 3: out_partial = attnT_norm^T @ Wo_g -> DRAM
Phase 2 is exp(ScalarE)-bound; leftover qT projection and phase-3 matmuls are
micro-interleaved into phase 2's per-iteration slack ("fillers") to keep the
TensorEngine busy and HAM-warm.
"""
import numpy as np
import concourse.bacc as bacc
import concourse.mybir as mybir
from concourse.tile import TileContext
from concourse.bass_utils import run_bass_kernel_spmd

N_CORES = 8
P = 128
F = 1024            # query/context feature dim
KF = F // P         # 8 f-tiles
HD = 512            # head-group hidden dim (8 heads x 64)
HT = HD // P        # 4 h-tiles
TQ = TC = 2048
NH = 8              # heads per core
D = 64
CH = 1024           # tq chunk for phase 2/3
NCH = TQ // CH      # 2
TCT = TC // P       # 16 tc tiles
F32 = mybir.dt.float32
F32R = mybir.dt.float32r
BF16 = mybir.dt.bfloat16
PH2_DT = BF16       # qT/kT/vv/probs dtype
PH1_DT = BF16       # inputs/weights dtype (host pre-casts)

_NC_CACHE = None


def build_kernel(debug=False):
    nc = bacc.Bacc("TRN2", target_bir_lowering=False, debug=False, num_devices=N_CORES)

    qt_d = nc.dram_tensor("qt", [F, TQ], PH1_DT, kind="ExternalInput")   # query[b].T
    ct_d = nc.dram_tensor("ct", [F, TC], PH1_DT, kind="ExternalInput")   # context[b].T
    wq_d = nc.dram_tensor("wq", [F, HD], PH1_DT, kind="ExternalInput")   # pre-scaled 1/8
    wk_d = nc.dram_tensor("wk", [F, HD], PH1_DT, kind="ExternalInput")
    wv_d = nc.dram_tensor("wv", [F, HD], PH1_DT, kind="ExternalInput")
    wo_d = nc.dram_tensor("wo", [HD, 1024], PH1_DT, kind="ExternalInput")
    bq_d = nc.dram_tensor("bq", [HT, P], F32, kind="ExternalInput")      # pre-scaled 1/8
    bk_d = nc.dram_tensor("bk", [HT, P], F32, kind="ExternalInput")
    bv_d = nc.dram_tensor("bv", [1, HD], PH1_DT, kind="ExternalInput")
    out_d = nc.dram_tensor("out", [TQ, 1024], F32, kind="ExternalOutput")
    scr_d = nc.dram_tensor("scr", [NCH, NH, CH], F32)  # denom bounce for broadcast
    if debug:
        at_dump = nc.dram_tensor("at_dump", [NCH, P, HT, CH], F32, kind="ExternalOutput")
        pa_dump = nc.dram_tensor("pa_dump", [D + 1, CH], F32, kind="ExternalOutput")
        bc_dump = nc.dram_tensor("bc_dump", [D, CH], F32, kind="ExternalOutput")

    with TileContext(nc) as tc:
        with tc.tile_pool(name="consts", bufs=1) as consts, \
             tc.tile_pool(name="qkv", bufs=1) as qkv, \
             tc.tile_pool(name="wts", bufs=1) as wts, \
             tc.tile_pool(name="chunks", bufs=2) as chunks, \
             tc.tile_pool(name="wop", bufs=1) as wop, \
             tc.tile_pool(name="probs", bufs=8) as probs, \
             tc.tile_pool(name="attnsb", bufs=2) as attnsb, \
             tc.tile_pool(name="norm", bufs=2) as norm, \
             tc.tile_pool(name="outsb", bufs=2) as outsb, \
             tc.tile_pool(name="ps_proj", bufs=2, space="PSUM") as ps_proj, \
             tc.tile_pool(name="ps_scores", bufs=2, space="PSUM") as ps_scores, \
             tc.tile_pool(name="ps_attn", bufs=1, space="PSUM") as ps_attn:

            ones = consts.tile([1, P], PH1_DT)
            nc.vector.memset(ones, 1.0)
            bv_sb = consts.tile([1, HD], PH1_DT)
            nc.sync.dma_start(out=bv_sb, in_=bv_d[:, :])
            bq_sb = consts.tile([P, HT], F32)
            nc.sync.dma_start(out=bq_sb, in_=bq_d.rearrange("m p -> p m"))
            bk_sb = consts.tile([P, HT], F32)
            nc.sync.dma_start(out=bk_sb, in_=bk_d.rearrange("m p -> p m"))

            qT = qkv.tile([P, HT, TQ], PH2_DT)          # [h-dim, tq]
            kT = qkv.tile([P, HT, TC], PH2_DT)          # [h-dim, tc]
            vv = qkv.tile([P, TCT, NH, D + 1], PH2_DT)  # per head: [v | ones]
            nc.vector.memset(vv[:, :, :, D:D + 1], 1.0)

            wq = wts.tile([P, KF, HD], PH1_DT)
            nc.sync.dma_start(out=wq, in_=wq_d.rearrange("(k p) h -> p k h", p=P))
            wk = wts.tile([P, KF, HD], PH1_DT)
            nc.sync.dma_start(out=wk, in_=wk_d.rearrange("(k p) h -> p k h", p=P))
            wv = wts.tile([P, KF, HD], PH1_DT)
            nc.sync.dma_start(out=wv, in_=wv_d.rearrange("(k p) h -> p k h", p=P))
            wo = wop.tile([P, HT, 1024], PH1_DT)
            nc.sync.dma_start(out=wo, in_=wo_d.rearrange("(m p) o -> p m o", p=P))

            qt_r = qt_d.rearrange("(k p) t -> p k t", p=P)
            ct_r = ct_d.rearrange("(k p) t -> p k t", p=P)

            # ---------------- phase 1: K/V projections, then Q chunk 0 ----------
            for n in range(TC // 512):
                cc = chunks.tile([P, KF, 512], PH1_DT, tag="chunk")
                nc.sync.dma_start(out=cc, in_=ct_r[:, :, n * 512:(n + 1) * 512])
                for m in range(HT):
                    ps = ps_proj.tile([P, 512], F32, tag="ps")
                    for k in range(KF):
                        nc.tensor.matmul(ps, wk[:, k, m * P:(m + 1) * P], cc[:, k, :],
                                         start=(k == 0), stop=(k == KF - 1))
                    nc.vector.tensor_scalar_add(
                        out=kT[:, m, n * 512:(n + 1) * 512], in0=ps,
                        scalar1=bk_sb[:, m:m + 1])
                for tl in range(4):
                    i = n * 4 + tl
                    ps = ps_proj.tile([P, 512], F32, tag="ps")
                    for k in range(KF):
                        nc.tensor.matmul(ps, cc[:, k, tl * P:(tl + 1) * P], wv[:, k, :],
                                         start=(k == 0), stop=False)
                    nc.tensor.matmul(ps, ones[0:1, 0:P], bv_sb[0:1, :],
                                     start=False, stop=True)
                    nc.vector.tensor_copy(
                        out=vv[:, i, :, 0:D],
                        in_=ps.rearrange("p (h d) -> p h d", h=NH))

            def qproj_ops(n, qc):
                """Micro-op generator: project query chunk n (tq cols n*512..+512)."""
                for m in range(HT):
                    ps = ps_proj.tile([P, 512], F32, tag="ps")
                    for k in range(KF):
                        nc.tensor.matmul(ps, wq[:, k, m * P:(m + 1) * P], qc[:, k, :],
                                         start=(k == 0), stop=(k == KF - 1))
                        yield
                    nc.vector.tensor_scalar_add(
                        out=qT[:, m, n * 512:(n + 1) * 512], in0=ps,
                        scalar1=bq_sb[:, m:m + 1])
                    yield

            def outproj_ops(c, at):
                """Micro-op generator: out[tq chunk c] = attnT^T @ Wo -> DRAM."""
                for t in range(CH // P):
                    ot = outsb.tile([P, 1024], F32)
                    for o in range(2):
                        po = ps_proj.tile([P, 512], F32, tag="ps")
                        for m in range(HT):
                            nc.tensor.matmul(po, at[:, m, t * P:(t + 1) * P],
                                             wo[:, m, o * 512:(o + 1) * 512],
                                             start=(m == 0), stop=(m == HT - 1))
                            yield
                        nc.vector.tensor_copy(out=ot[:, o * 512:(o + 1) * 512], in_=po)
                        yield
                    nc.sync.dma_start(
                        out=out_d[c * CH + t * P:c * CH + (t + 1) * P, :], in_=ot)
                    yield

            # Q chunks 0,1 (tq 0..1023) emitted directly before phase 2
            for n in range(2):
                qc = chunks.tile([P, KF, 512], PH1_DT, tag="chunk")
                nc.sync.dma_start(out=qc, in_=qt_r[:, :, n * 512:(n + 1) * 512])
                for _ in qproj_ops(n, qc):
                    pass

            # ---------------- phase 2 + 3 (with fillers) ----------------
            at_prev = None
            for c in range(NCH):
                if c == 0:
                    # prefetch + filler-project Q chunks 2,3 during chunk 0's attention
                    qc2 = chunks.tile([P, KF, 512], PH1_DT, tag="chunk")
                    nc.sync.dma_start(out=qc2, in_=qt_r[:, :, 2 * 512:3 * 512])
                    qc3 = chunks.tile([P, KF, 512], PH1_DT, tag="chunk")
                    nc.sync.dma_start(out=qc3, in_=qt_r[:, :, 3 * 512:4 * 512])
                    fillers = _chain(qproj_ops(2, qc2), qproj_ops(3, qc3))
                else:
                    fillers = outproj_ops(c - 1, at_prev)

                at = attnsb.tile([P, HT, CH], PH1_DT, tag="attnsb")
                for h in range(NH):
                    ht, hp = h // 2, (h % 2) * D
                    pa = ps_attn.tile([D + 1, CH], F32)
                    for i in range(TCT):
                        ss = ps_scores.tile([P, CH], F32)
                        for n in range(CH // 512):
                            nc.tensor.matmul(
                                ss[:, n * 512:(n + 1) * 512],
                                kT[hp:hp + D, ht, i * P:(i + 1) * P],
                                qT[hp:hp + D, ht,
                                   c * CH + n * 512:c * CH + (n + 1) * 512],
                                start=True, stop=True)
                        pb = probs.tile([P, CH], PH2_DT)
                        nc.scalar.activation(pb, ss, mybir.ActivationFunctionType.Exp)
                        for n in range(CH // 512):
                            nc.tensor.matmul(
                                pa[:, n * 512:(n + 1) * 512],
                                vv[:, i, h, :],
                                pb[:, n * 512:(n + 1) * 512],
                                start=(i == 0), stop=(i == TCT - 1))
                        next(fillers, None)  # one filler micro-op per iteration
                    # normalize: evacuate PSUM promptly (frees pa for next head),
                    # approx-recip of denom row broadcast via DRAM bounce, multiply
                    pa_sb = norm.tile([D + 1, CH], F32, tag="pasb")
                    nc.vector.tensor_copy(out=pa_sb, in_=pa)
                    if debug and c == 0 and h == 0:
                        nc.sync.dma_start(out=pa_dump[:, :], in_=pa_sb)
                    nc.sync.dma_start(out=scr_d[c, h:h + 1, :], in_=pa_sb[D:D + 1, :])
                    dn = norm.tile([D, CH], F32, tag="dn")
                    nc.gpsimd.dma_start(
                        out=dn, in_=scr_d[c, h:h + 1, :].to_broadcast([D, CH]))
                    bc = norm.tile([D, CH], F32, tag="bc")
                    nc.vector.reciprocal_approx_fast(out=bc, in_=dn)
                    if debug and c == 0 and h == 0:
                        nc.sync.dma_start(out=bc_dump[:, :], in_=bc)
                    if h % 2 == 0:
                        nc.vector.tensor_mul(out=at[0:D, ht, :], in0=pa_sb[0:D, :], in1=bc)
                    else:
                        tmp = norm.tile([D, CH], PH1_DT, tag="tmp")
                        nc.vector.tensor_mul(out=tmp, in0=pa_sb[0:D, :], in1=bc)
                        nc.gpsimd.dma_start(out=at[D:P, ht, :], in_=tmp)
                for _ in fillers:  # drain any leftovers
                    pass
                if debug:
                    atf = attnsb.tile([P, HT, CH], F32, tag="at_dbg")
                    nc.vector.tensor_copy(out=atf, in_=at)
                    nc.sync.dma_start(out=at_dump[c], in_=atf)
                at_prev = at
            # final chunk's output projection (tail)
            for _ in outproj_ops(NCH - 1, at_prev):
                pass

    nc.compile()
    return nc


def _chain(*gens):
    for g in gens:
        yield from g


def make_in_maps(query, context, Wq, bq, Wk, bk, Wv, bv, Wo, bo):
    import ml_dtypes
    cast1 = (lambda a: np.asarray(a, np.float32)) if PH1_DT == F32R \
        else (lambda a: np.asarray(a, np.float32).astype(ml_dtypes.bfloat16))
    query = np.asarray(query, np.float32)
    context = np.asarray(context, np.float32)
    Wq = np.asarray(Wq, np.float32); bq = np.asarray(bq, np.float32)
    Wk = np.asarray(Wk, np.float32); bk = np.asarray(bk, np.float32)
    Wv = np.asarray(Wv, np.float32); bv = np.asarray(bv, np.float32)
    Wo = np.asarray(Wo, np.float32)

    in_maps = []
    for c in range(N_CORES):
        b, g = c // 2, c % 2
        sl = slice(g * HD, (g + 1) * HD)
        in_maps.append({
            "qt": cast1(np.ascontiguousarray(query[b].T)),
            "ct": cast1(np.ascontiguousarray(context[b].T)),
            "wq": cast1(np.ascontiguousarray(Wq[:, sl] * 0.125)),
            "wk": cast1(np.ascontiguousarray(Wk[:, sl])),
            "wv": cast1(np.ascontiguousarray(Wv[:, sl])),
            "wo": cast1(np.ascontiguousarray(Wo[sl, :])),
            "bq": np.ascontiguousarray((bq[sl] * 0.125).reshape(HT, P)),
            "bk": np.ascontiguousarray(bk[sl].reshape(HT, P)),
            "bv": cast1(bv[sl].reshape(1, HD)),
        })
    return in_maps


def kernel(query, context, Wq, bq, Wk, bk, Wv, bv, Wo, bo):
    global _NC_CACHE
    if _NC_CACHE is None:
        _NC_CACHE = build_kernel()
    nc = _NC_CACHE
    bo = np.asarray(bo, np.float32)

    in_maps = make_in_maps(query, context, Wq, bq, Wk, bk, Wv, bv, Wo, bo)
    res = run_bass_kernel_spmd(nc, in_maps, list(range(N_CORES)))
    out = np.empty((4, TQ, 1024), np.float32)
    for b in range(4):
        out[b] = res.results[2 * b]["out"] + res.results[2 * b + 1]["out"] + bo
    return out
